# revision 14
# baseline (speedup 1.0000x reference)
import math
import os
import sys

import numpy as np

for _p in ("/opt/trn_rl_repo", "/root/.axon_site/_ro/trn_rl_repo"):
    if os.path.isdir(_p) and _p not in sys.path:
        sys.path.insert(0, _p)

import concourse.bacc as bacc
import concourse.bass as bass
import concourse.tile as tile
from concourse import mybir
from concourse.bass_utils import run_bass_kernel_spmd

F32 = mybir.dt.float32
F16 = mybir.dt.float16
AF = mybir.ActivationFunctionType
OP = mybir.AluOpType

# ---- problem constants (hardcoded; kernel.py must be self-contained) ----
RANGES_MIN = np.array([170., 85000., -110., -80., 170., 0., -110., -100., -1000.], np.float64)
RANGES_MAX = np.array([350., 110000., 110., 80., 350., 0.04, 110., 100., 60000.], np.float64)
MS_WEIGHTS = np.array([0.0448, 0.2856, 0.3001, 0.2363, 0.1333], np.float64)
C1 = 0.01 ** 2
C2 = 0.03 ** 2
NVARS, NLEV, H0, W0 = 9, 13, 721, 1440
NCH = NVARS * NLEV        # 117
NCORES = 8
CH = 15                   # channels per core (8*15 = 120, last 3 padded)

# per-scale geometry: (H, W, T storage tiles, Ws strips, Wpad)
def scale_dims():
    dims = []
    h, w = H0, W0
    for s in range(5):
        hc, wc = h - 10, w - 10
        t = 1 if h <= 128 else (h - 128 + 117) // 118 + 1
        ws = (wc + 117) // 118
        wpad = 118 * (ws - 1) + 128
        wpad = max(wpad, w)
        dims.append((h, w, hc, wc, t, ws, wpad))
        h = (h + (h % 2) * 2 - 2) // 2 + 1 if False else (h + 2 * (h % 2)) // 2
        w = (w + 2 * (w % 2)) // 2
    return dims

SD = scale_dims()   # [(721,1440,711,1430,7,13,1544), (361,720,...), ...]

# acc column layout (per channel slot): cs strips per scale, ssim(s4), pixel tiles
CS_COLS = [sd[5] for sd in SD]            # 13,7,3,2,1
NCS = sum(CS_COLS)                        # 26
COL_SSIM = NCS                            # 26
COL_PIX = NCS + 1                         # 27..33 (7 tiles)
NACC = COL_PIX + SD[0][4]                 # 34


def gauss_win():
    c = np.arange(11, dtype=np.float64) - 5.0
    g = np.exp(-(c * c) / (2 * 1.5 * 1.5))
    return g / g.sum()


def gauss_win_f16():
    """fp16 window nudged by ulps so the fp16 taps sum to exactly 1.0
    (the raw-rounded sum is off by 1.6e-4, which systematically biases
    the SSIM covariance cancellation)."""
    f16 = np.float16
    w16 = gauss_win().astype(f16)
    for _ in range(200):
        r = 1.0 - w16.astype(np.float64).sum()
        if abs(r) < 1e-7:
            break
        best, bi = None, None
        for i in range(11):
            up = np.nextafter(w16[i], f16(np.inf) if r > 0 else f16(-np.inf))
            step = float(up) - float(w16[i])
            if abs(step) <= abs(r) * 1.5 and (best is None or abs(step) > abs(best)):
                best, bi = step, i
        if bi is None:
            break
        w16[bi] = np.nextafter(w16[bi], f16(np.inf) if r > 0 else f16(-np.inf))
    return w16.astype(np.float64)


def build_band():
    win = gauss_win_f16()
    b = np.zeros((128, 118), np.float32)
    for m in range(118):
        b[m:m + 11, m] = win
    return b


def build_pool_mats():
    """Pool matrices per scale transition: list of (trans, t_out, q_in, mat128x128)."""
    mats = []
    for s in range(4):
        hin, tin = SD[s][0], SD[s][4]
        hout, tout = SD[s + 1][0], SD[s + 1][4]
        for tp in range(tout):
            byq = {}
            for j in range(128):
                J = 118 * tp + j
                if J >= hout:
                    continue
                for r in (2 * J - 1, 2 * J):
                    if 0 <= r < hin:
                        q = min(r // 118, tin - 1)
                        byq.setdefault(q, np.zeros((128, 128), np.float32))[r - 118 * q, j] += 0.25
            for q in sorted(byq):
                mats.append((s, tp, q, byq[q]))
    return mats


POOL_MATS = build_pool_mats()
NPM = len(POOL_MATS)


PH_E = True      # phase E (load/norm/pixel)
PH_C1 = True     # pass1 + copy
PH_C2 = True     # pass2 + cs
PH_P = True      # pooling
PH_SMAX = 5      # scales 0..PH_SMAX-1
PH_CS = 9        # cs chain depth: 1=mm,2=+sq,3=+P/Q,4=+B2/recip,5=+ttr


def build_program(ch=CH):
    nc = bacc.Bacc("TRN2", target_bir_lowering=False, debug=False, num_devices=NCORES)
    x_d = nc.dram_tensor("x", [ch, H0, W0], F32, kind="ExternalInput").ap()
    y_d = nc.dram_tensor("y", [ch, H0, W0], F32, kind="ExternalInput").ap()
    band_d = nc.dram_tensor("band", [128, 118], F16, kind="ExternalInput").ap()
    pm_d = nc.dram_tensor("poolmats", [NPM, 128, 128], F16, kind="ExternalInput").ap()
    nrm_d = nc.dram_tensor("normc", [ch, 2], F32, kind="ExternalInput").ap()
    acc_d = nc.dram_tensor("acc", [128, ch * NACC], F32, kind="ExternalOutput").ap()

    with tile.TileContext(nc) as tc:
        import contextlib
        ctx = contextlib.ExitStack()
        singles = ctx.enter_context(tc.tile_pool(name="singles", bufs=1))
        iop = ctx.enter_context(tc.tile_pool(name="io", bufs=2))
        imgp = ctx.enter_context(tc.tile_pool(name="img", bufs=1))
        pixp = ctx.enter_context(tc.tile_pool(name="pix", bufs=2))
        pix1 = ctx.enter_context(tc.tile_pool(name="pix1", bufs=1))
        o1p = ctx.enter_context(tc.tile_pool(name="o1", bufs=5))
        sqp = ctx.enter_context(tc.tile_pool(name="sq", bufs=3))
        csp = ctx.enter_context(tc.tile_pool(name="cs", bufs=2))
        cs1 = ctx.enter_context(tc.tile_pool(name="cs1", bufs=1))
        cs4 = ctx.enter_context(tc.tile_pool(name="cs4", bufs=1))
        ps1 = ctx.enter_context(tc.tile_pool(name="ps1", bufs=1, space="PSUM"))
        ps2 = ctx.enter_context(tc.tile_pool(name="ps2", bufs=2, space="PSUM"))
        psp = ctx.enter_context(tc.tile_pool(name="psp", bufs=2, space="PSUM"))

        band = singles.tile([128, 118], F16)
        nc.sync.dma_start(out=band, in_=band_d)
        pmats = singles.tile([128, NPM, 128], F16)
        nc.sync.dma_start(out=pmats, in_=pm_d.rearrange("n p w -> p n w"))
        nrm = singles.tile([128, ch * 2], F32)
        nc.sync.dma_start(
            out=nrm,
            in_=bass.AP(tensor=nrm_d.tensor, offset=nrm_d.offset,
                        ap=[[0, 128], [1, ch * 2]]),
        )
        acc = singles.tile([128, ch * NACC], F32)
        nc.vector.memset(acc, 0.0)
        dummy = singles.tile([128, 1], F32)
        dummy2 = singles.tile([128, 1], F32)

        # persistent fp16 image storage per scale (S and D)
        sbufs, dbufs = [], []
        for s, (h, w, hc, wc, t, ws, wpad) in enumerate(SD):
            sbufs.append(imgp.tile([128, t, wpad], F16, tag=f"S{s}", name=f"S{s}"))
            dbufs.append(imgp.tile([128, t, wpad], F16, tag=f"D{s}", name=f"D{s}"))

        for c in range(ch):
            # ---------------- phase E: load + normalize + pixel loss + S/D ----
            h, w, hc, wc, T, Ws, wpad = SD[0]
            S0, D0 = sbufs[0], dbufs[0]
            for t in range(T):
                r0 = 118 * t
                rows = min(128, h - r0)
                xt = iop.tile([128, w], F32, tag="xt")
                yt = iop.tile([128, w], F32, tag="yt")
                if rows < 128:
                    nc.gpsimd.memset(xt, 0.0)
                    nc.gpsimd.memset(yt, 0.0)
                nc.sync.dma_start(out=xt[0:rows, :], in_=x_d[c, r0:r0 + rows, :])
                nc.sync.dma_start(out=yt[0:rows, :], in_=y_d[c, r0:r0 + rows, :])
                # clip((v-lo)/span, 0, 1) = relu(1 - relu(1 - (a*v+b)))
                sc = nrm[:, 2 * c:2 * c + 1]        # -a
                bi = nrm[:, 2 * c + 1:2 * c + 2]    # 1-b
                xr = pixp.tile([128, w], F32, tag="xr")
                yr = pixp.tile([128, w], F32, tag="yr")
                nc.scalar.activation(xr, xt, AF.Relu, bias=bi, scale=sc)
                nc.scalar.activation(xr, xr, AF.Relu, bias=1.0, scale=-1.0)
                nc.scalar.activation(yr, yt, AF.Relu, bias=bi, scale=sc)
                nc.scalar.activation(yr, yr, AF.Relu, bias=1.0, scale=-1.0)
                d = pixp.tile([128, w], F32, tag="d")
                nc.vector.tensor_sub(d, xr, yr)
                nc.vector.tensor_add(S0[:, t, 0:w], xr, yr)
                nc.vector.tensor_copy(D0[:, t, 0:w], d)
                # pixel loss on valid rows only (in-place chains on scratch)
                if not PH_E:
                    continue
                pv = min(118, h - r0)
                t_ad = pix1.tile([128, w], F32, tag="t_ad")
                t_d2 = pix1.tile([128, w], F32, tag="t_d2")
                t_w = pix1.tile([128, w], F32, tag="t_w")
                nc.vector.scalar_tensor_tensor(t_ad[0:pv], d[0:pv], -1.0, d[0:pv], OP.mult, OP.max)
                nc.gpsimd.tensor_mul(t_d2[0:pv], d[0:pv], d[0:pv])
                nc.scalar.activation(t_w[0:pv], yr[0:pv], AF.Square)
                nc.vector.tensor_mul(t_w[0:pv], t_w[0:pv], yr[0:pv])
                nc.scalar.activation(t_w[0:pv], t_w[0:pv], AF.Exp, bias=0.0, scale=5.0)
                nc.vector.scalar_tensor_tensor(t_ad[0:pv], t_w[0:pv], 1.0, t_ad[0:pv], OP.add, OP.mult)
                nc.gpsimd.tensor_mul(t_d2[0:pv], t_d2[0:pv], t_w[0:pv])
                nc.vector.scalar_tensor_tensor(
                    t_ad[0:pv], t_ad[0:pv], 1.0, t_d2[0:pv], OP.mult, OP.subtract,
                    accum_out=acc[0:pv, c * NACC + COL_PIX + t: c * NACC + COL_PIX + t + 1])
            nc.gpsimd.memset(S0[:, :, w:wpad], 0.0)
            nc.gpsimd.memset(D0[:, :, w:wpad], 0.0)

            # ---------------- per-scale conv + cs ----------------------------
            cs_col0 = 0
            for s, (h, w, hc, wc, T, Ws, wpad) in enumerate(SD):
                if s >= PH_SMAX:
                    break
                S, D = sbufs[s], dbufs[s]
                th = (hc + 117) // 118
                for ws_i in range(Ws if PH_C1 else 0):
                    c0 = 118 * ws_i
                    pvw = min(118, wc - c0)
                    # pass 1 (fused transpose + vertical conv), 4 images
                    o1 = {}
                    for im in range(4):
                        p1 = ps1.tile([128, th, 128], F32, tag="p1")
                        for t in range(th):
                            if im == 0:
                                lhsT = S[:, t, c0:c0 + 128]
                            elif im == 1:
                                lhsT = D[:, t, c0:c0 + 128]
                            else:
                                src = S if im == 2 else D
                                sq = sqp.tile([128, 128], F16, tag="sq")
                                nc.vector.tensor_mul(sq, src[:, t, c0:c0 + 128],
                                                     src[:, t, c0:c0 + 128])
                                lhsT = sq
                            nc.tensor.matmul(p1[:, t, 0:118], lhsT, band,
                                             start=True, stop=True)
                        o1t = o1p.tile([128, 896], F16, tag="o1")
                        if im % 2 == 0:
                            nc.vector.tensor_copy(o1t[:, 0:th * 118], p1[:, :, 0:118])
                        else:
                            nc.scalar.copy(o1t[:, 0:th * 118], p1[:, :, 0:118])
                        o1[im] = o1t
                    # pass 2 (stationary band horizontal conv) + cs chain
                    if not PH_C2:
                        continue
                    p2 = {}
                    for im in range(4):
                        pt = ps2.tile([118, 1024], F32, tag="p2")
                        n0 = 0
                        while n0 < hc:
                            nn = min(512, hc - n0)
                            nc.tensor.matmul(pt[:, n0:n0 + nn], band,
                                             o1[im][:, n0:n0 + nn], start=True, stop=True)
                            n0 += nn
                        p2[im] = pt
                        if PH_CS < 2:
                            continue
                        if im == 0:
                            s1v = csp.tile([128, 1024], F32, tag="s1v")
                            nc.scalar.activation(s1v[0:pvw, 0:hc], pt[0:pvw, 0:hc], AF.Square)
                        elif im == 1:
                            s2v = csp.tile([128, 1024], F32, tag="s2v")
                            nc.scalar.activation(s2v[0:pvw, 0:hc], pt[0:pvw, 0:hc], AF.Square)
                    if PH_CS < 3:
                        continue
                    p2t = cs1.tile([128, 1024], F32, tag="p2t")
                    nc.vector.scalar_tensor_tensor(
                        p2t[0:pvw, 0:hc], p2[2][0:pvw, 0:hc], 2 * C2, s1v[0:pvw, 0:hc],
                        OP.add, OP.subtract)
                    qt = cs1.tile([128, 1024], F32, tag="qt")
                    nc.vector.scalar_tensor_tensor(
                        qt[0:pvw, 0:hc], p2[3][0:pvw, 0:hc], 0.0, s2v[0:pvw, 0:hc],
                        OP.add, OP.subtract)
                    if PH_CS < 4:
                        continue
                    b2 = cs1.tile([128, 1024], F32, tag="b2")
                    nc.vector.tensor_add(b2[0:pvw, 0:hc], p2t[0:pvw, 0:hc], qt[0:pvw, 0:hc])
                    nc.scalar.activation(b2[0:pvw, 0:hc], b2[0:pvw, 0:hc], AF.Ln)
                    nc.scalar.activation(b2[0:pvw, 0:hc], b2[0:pvw, 0:hc], AF.Exp,
                                         bias=0.0, scale=-1.0)
                    if PH_CS < 5:
                        continue
                    col = c * NACC + cs_col0 + ws_i
                    nc.vector.tensor_mul(p2t[0:pvw, 0:hc], qt[0:pvw, 0:hc], b2[0:pvw, 0:hc])
                    nc.vector.tensor_reduce(
                        acc[0:pvw, col:col + 1], p2t[0:pvw, 0:hc],
                        axis=mybir.AxisListType.X, op=OP.add)
                    if s == 4:
                        # ssim = l * cs ; l = (s1v - s2v + 2C1)/(s1v + s2v + 2C1)
                        ut = cs4.tile([128, 64], F32, tag="ut")
                        nc.vector.scalar_tensor_tensor(
                            ut[0:pvw, 0:hc], s1v[0:pvw, 0:hc], 2 * C1, s2v[0:pvw, 0:hc],
                            OP.add, OP.subtract)
                        vt = cs4.tile([128, 64], F32, tag="vt")
                        nc.vector.scalar_tensor_tensor(
                            vt[0:pvw, 0:hc], s1v[0:pvw, 0:hc], 2 * C1, s2v[0:pvw, 0:hc],
                            OP.add, OP.add)
                        nc.scalar.activation(vt[0:pvw, 0:hc], vt[0:pvw, 0:hc], AF.Ln)
                        nc.scalar.activation(vt[0:pvw, 0:hc], vt[0:pvw, 0:hc], AF.Exp,
                                             bias=0.0, scale=-1.0)
                        nc.vector.tensor_mul(ut[0:pvw, 0:hc], ut[0:pvw, 0:hc], vt[0:pvw, 0:hc])
                        cst = cs4.tile([128, 64], F32, tag="cst")
                        nc.vector.tensor_scalar(cst[0:pvw, 0:hc], p2t[0:pvw, 0:hc],
                                                -2.0, 1.0, OP.mult, OP.add)
                        lcs = cs4.tile([128, 64], F32, tag="lcs")
                        colm = c * NACC + COL_SSIM
                        nc.vector.tensor_mul(lcs[0:pvw, 0:hc], ut[0:pvw, 0:hc], cst[0:pvw, 0:hc])
                        nc.vector.tensor_reduce(
                            acc[0:pvw, colm:colm + 1], lcs[0:pvw, 0:hc],
                            axis=mybir.AxisListType.X, op=OP.add)
                cs_col0 += Ws

                # ------------- pool to next scale ---------------------------
                if s < 4 and PH_P:
                    hn, wn_, hcn, wcn, Tn, Wsn, wpadn = SD[s + 1]
                    Sn, Dn = sbufs[s + 1], dbufs[s + 1]
                    trans = [(tp, q, i) for i, (ts_, tp, q, _) in enumerate(POOL_MATS)
                             if ts_ == s]
                    byt = {}
                    for tp, q, i in trans:
                        byt.setdefault(tp, []).append((q, i))
                    for src, dst in ((S, Sn), (D, Dn)):
                        for tp, qs in byt.items():
                            w0c = 0
                            while w0c < w:
                                wnn = min(512, w - w0c)
                                pp = psp.tile([128, 512], F32, tag="pp")
                                for k, (q, i) in enumerate(qs):
                                    nc.tensor.matmul(
                                        pp[:, 0:wnn], pmats[:, i, :],
                                        src[:, q, w0c:w0c + wnn],
                                        start=(k == 0), stop=(k == len(qs) - 1))
                                with nc.allow_low_precision(reason="2-elem pool pair add to fp16"):
                                    nc.vector.tensor_reduce(
                                        dst[:, tp, w0c // 2:(w0c + wnn) // 2],
                                        pp[:, 0:wnn].rearrange("p (a b) -> p a b", b=2),
                                        axis=mybir.AxisListType.X, op=OP.add)
                                w0c += wnn
                        nc.gpsimd.memset(dst[:, :, wn_:wpadn], 0.0)

        nc.sync.dma_start(out=acc_d, in_=acc)
        ctx.close()
    nc.compile()
    return nc


def host_inputs(x, y, ch=CH):
    """Build per-core in_maps from full inputs."""
    xf = x.reshape(NCH, H0, W0)
    yf = y.reshape(NCH, H0, W0)
    pad = NCORES * ch - NCH
    if pad > 0:
        xf = np.concatenate([xf, np.zeros((pad, H0, W0), np.float32)], 0)
        yf = np.concatenate([yf, np.zeros((pad, H0, W0), np.float32)], 0)
    band = build_band().astype(np.float16)
    pm = np.stack([m for (_, _, _, m) in POOL_MATS]).astype(np.float16)
    lo = RANGES_MIN.repeat(NLEV)
    span = (RANGES_MAX - RANGES_MIN).repeat(NLEV)
    a = 1.0 / span
    b = -lo / span
    normc_all = np.stack([-a, 1.0 - b], 1).astype(np.float32)
    if pad > 0:
        normc_all = np.concatenate([normc_all, np.tile(normc_all[-1:], (pad, 1))], 0)
    in_maps = []
    for core in range(NCORES):
        sl = slice(core * ch, core * ch + ch)
        in_maps.append({
            "x": np.ascontiguousarray(xf[sl]),
            "y": np.ascontiguousarray(yf[sl]),
            "band": band, "poolmats": pm,
            "normc": np.ascontiguousarray(normc_all[sl]),
        })
    return in_maps


def host_combine(accs, ch=CH):
    """accs: list of [128, ch*NACC] per core -> scalar loss (f64)."""
    cs_mean = np.zeros((NCORES * ch, 5))
    ssim_mean = np.zeros(NCORES * ch)
    pix_sum = np.zeros(NCORES * ch)
    for core in range(NCORES):
        a = accs[core].reshape(128, ch, NACC).astype(np.float64)
        for sl in range(ch):
            g = core * ch + sl
            col0 = 0
            for s, (h, w, hc, wc, T, Ws, wpad) in enumerate(SD):
                tot = 0.0
                for wsi in range(Ws):
                    pvw = min(118, wc - 118 * wsi)
                    tot += a[0:pvw, sl, col0 + wsi].sum()
                cs_mean[g, s] = 1.0 - 2.0 * tot / (hc * wc)
                col0 += Ws
            hc4, wc4 = SD[4][2], SD[4][3]
            ssim_mean[g] = a[0:wc4, sl, COL_SSIM].sum() / (hc4 * wc4)
            for t in range(SD[0][4]):
                pv = min(118, H0 - 118 * t)
                pix_sum[g] += a[0:pv, sl, COL_PIX + t].sum()
    cs_mean = cs_mean[:NCH]
    ssim_mean = ssim_mean[:NCH]
    pix_sum = pix_sum[:NCH]
    vals = np.concatenate([np.maximum(cs_mean[:, :4], 0.0),
                           np.maximum(ssim_mean, 0.0)[:, None]], 1)
    ms = np.prod(vals ** MS_WEIGHTS[None, :], 1).mean()
    pixel_loss = 0.5 * pix_sum.sum() / (NCH * H0 * W0)
    return (1.0 - ms) + pixel_loss


_NC_CACHE = {}


def kernel(x: np.ndarray, y: np.ndarray) -> np.ndarray:
    ch = CH
    if ch not in _NC_CACHE:
        _NC_CACHE[ch] = build_program(ch)
    nc = _NC_CACHE[ch]
    in_maps = host_inputs(x, y, ch)
    res = run_bass_kernel_spmd(nc, in_maps, list(range(NCORES)))
    accs = [res.results[i]["acc"] for i in range(NCORES)]
    out = host_combine(accs, ch)
    return np.float32(out)
